# revision 20
# baseline (speedup 1.0000x reference)
"""GCNEncoder Trainium2 kernel (8 NeuronCores, SPMD).

Strategy (graph/data parallel, per sharding hint):
  - Nodes are dealt round-robin-by-degree across 8 cores (2500 each); the
    [H,H] weights are replicated.
  - Layer-0's gather table (bf16(dinv * x)) is a function of the INPUT, so
    the host stages it directly into each core's HBM: no AllGather, no x
    load, no on-chip prologue for layer 1.
  - Per remaining GCN layer: each core's dinv-scaled bf16 output tiles are
    AllGathered into every core's HBM (one collective per table; only two
    AllGathers total are exposed at the layer boundaries).
  - Message aggregation = segment-sum over in-edges:  per 128-destination
    group, a transposed dma_gather pulls the source rows (feature-major:
    [128h, 2, 128*K]) and a fold-then-halve DVE cascade sums each
    destination's K slots (padding slots point at an all-zero table row).
  - Self-loop contributions never touch the gather: each destination's own
    row already sits in SBUF (the tt tile the core just produced); a PE
    transpose (identity matmul) + ACT copy stages it as an extra matmul
    accumulated into the same PSUM tile as the gathered aggregate.
  - The GCNConv reorder agg(x) @ W == agg(x @ W) lets one aggregation per
    layer feed the [HxH] matmul afterwards; out2/out3 share the layer-3
    aggregation.  norm = dinv[row]*dinv[col] factorizes into the table
    pre-scale and a per-destination post-scale fused into the PSUM->SBUF
    activation (bias is added via a K=1 rank-1 matmul of sqrt(deg) x b).
  - Gather chunk boundaries come from a per-128-group DP that trades slot
    padding (1.42 ns/slot of serial gather DMA) against per-chunk fixed
    cost; structural constants (dinv, sqrt(deg), bf16 weights, identity)
    are staged from the host.

Self-contained: hardcodes the problem shapes; only needs numpy + concourse.
"""

import os

import numpy as np

# Defensive: a previous process dying mid-run can leave /dev/neuron* wedged
# (NRT_EXEC_UNIT_UNRECOVERABLE on the next open). Asking NRT to reset cores
# at init recovers it; must be set before the first jax/PJRT init.
os.environ.setdefault("NEURON_RT_RESET_CORES", "1")

# -------------------- problem constants --------------------
N_NODES = 20000
N_EDGES = 320000
H = 256
C = 8  # cores

MAXI = 2944  # max gather indices per dma_gather (descriptor-ring limit)
DP_LAM = 100.0
DP_GAMMA = 0.3

# table layout: per-rank stripe of PT dest rows + 16 zero-pad rows
P_CONST = N_NODES // C          # 2500
PT_CONST = ((P_CONST + 127) // 128) * 128  # 2560
NG_CONST = PT_CONST // 128      # 20
RR = P_CONST + 16               # 2516 rows per rank (incl zpad)
NTAB = C * RR                   # 20128 table rows
ZROW = P_CONST                  # 2500: rank0's first zero-pad row

_KERNEL_CACHE = {}
LAST_RESULTS = None  # BassKernelResults of the most recent run (for profiling)


def _trow(n, P):
    """Table row of new node id n (vectorized)."""
    n = np.asarray(n)
    return (n // P) * RR + (n % P)


def _dve_cost(K):
    """DVE ns per destination for a K-slot chunk: fold-odd-then-halve
    cascade (2x mode) + a short strided-add tail into the matmul lhsT."""
    if K == 1:
        return 1.04
    c, cost = K, 0.0
    while c > 3:
        if c % 2:
            cost += 1.04  # fold the odd slot into slot 0
            c -= 1
        cost += 1.04 * (c // 2)
        c //= 2
    return cost + max(2.08 * (c - 1), 2.08)


# -------------------- host-side graph prep --------------------
def _prep_graph(edge_index, n_nodes, n_cores):
    """Partition nodes, build per-core padded gather-slot index arrays.

    Returns dict with permutation, per-core degree arrays, gather indices.
    """
    P = n_nodes // n_cores  # nodes per core
    row = edge_index[0].astype(np.int64)
    col = edge_index[1].astype(np.int64)
    deg = np.bincount(col, minlength=n_nodes).astype(np.int64) + 1  # incl loop

    # deal nodes round-robin by ascending degree -> every core gets an
    # almost identical degree profile, sorted ascending within the core.
    order = np.argsort(deg, kind="stable")
    pos = np.empty(n_nodes, dtype=np.int64)
    pos[order] = np.arange(n_nodes)
    # descending degree within the core: the zero-pad dests (local ids >= P)
    # then share a gather chunk with the LOWEST-degree nodes, so their K (and
    # wasted slots) is small instead of the global max
    new_id = (pos % n_cores) * P + (P - 1 - pos // n_cores)  # old -> new
    orig_of_new = np.empty(n_nodes, dtype=np.int64)
    orig_of_new[new_id] = np.arange(n_nodes)

    # self-loops are folded in on-chip; only real edges gather
    src_new = new_id[row]
    dst_new = new_id[col]

    PT = ((P + 127) // 128) * 128  # padded dest count per core
    NG = PT // 128  # 128-dest groups

    deg_new = deg[orig_of_new]  # per new id (includes the +1 self loop)
    k_new = deg_new - 1  # gather slots actually needed per dest

    # per-core padded arrays
    deg_loc = np.ones((n_cores, PT), dtype=np.float32)
    k_loc = np.zeros((n_cores, PT), dtype=np.int64)
    for c in range(n_cores):
        deg_loc[c, :P] = deg_new[c * P : (c + 1) * P]
        k_loc[c, :P] = k_new[c * P : (c + 1) * P]

    # max (over cores) slot count within a local-dest range (raw, unrounded)
    def range_K(lo, hi):
        m = 0
        for c in range(n_cores):
            seg = k_loc[c, lo:hi]
            if seg.size:
                m = max(m, int(seg.max()))
        return m

    # Chunking: per 128-dest group, DP over split points (32-dest
    # granularity keeps n_idx%128 with K%4) minimizing joint serial cost:
    #   1.42*slots (gather DMA) + 0.7*dve_cost (co-critical DVE reduce)
    #   + LAM_NS per chunk,   subject to dc*K <= MAXI.
    # The chunk may also round K UP past the minimum when a larger K has a
    # cheaper DVE-reduce tail.
    LAM_NS = DP_LAM  # per-chunk fixed cost
    GAMMA = DP_GAMMA  # DVE-reduce weight vs gather-DMA in the joint chunk cost

    def seg_cost(dc, mink):
        # K granularity follows the n_idx%128 transpose-gather constraint
        if mink == 0:
            # all-pad segment: no gather at all — the kernel memsets the
            # matmul-lhsT columns instead (sentinel K=0)
            return (0.0, 0)
        # dc*K must be %128: with dc = 16*m, K must be %(8//gcd(m,8))
        import math as _math
        step = 8 // _math.gcd(dc // 16, 8)
        base = ((mink + step - 1) // step) * step
        best = None
        for cK in range(base, base + 4 * step, step):
            if dc * cK > MAXI:
                continue
            c = dc * (1.42 * cK + GAMMA * _dve_cost(cK))
            if best is None or c < best[0]:
                best = (c, cK)
        return best

    chunks = []  # (dest_off_in_core, dc, K, idx_off)
    Kg = []
    ioff = 0
    STEP = 16
    for g in range(NG):
        base = g * 128
        nseg = 128 // STEP
        INF = float("inf")
        dp = [INF] * (nseg + 1)
        dp[0] = 0.0
        prev = [(0, 4)] * (nseg + 1)
        for i in range(1, nseg + 1):
            for j in range(i):
                dc = (i - j) * STEP
                mink = range_K(base + j * STEP, base + i * STEP)
                sc = seg_cost(dc, mink)
                if sc is None:
                    continue
                cost = dp[j] + sc[0] + LAM_NS
                if cost < dp[i]:
                    dp[i] = cost
                    prev[i] = (j, sc[1])
        # walk back
        cuts = []
        i = nseg
        while i > 0:
            j, cK = prev[i]
            cuts.append((j, i, cK))
            i = j
        gK = 4
        for j, i, cK in reversed(cuts):
            dc = (i - j) * STEP
            if cK == 0:
                continue  # all-pad: kernel memsets these Rb columns
            assert (dc * cK) % 128 == 0 and dc * cK <= MAXI
            chunks.append([base + j * STEP, dc, cK, ioff, 0])
            ioff += dc * cK
            gK = max(gK, cK)
        Kg.append(gK)
    TOT = int(ioff)  # slots per core (same for all cores)

    # NOTE: tensor work must stay off the Pool engine — it issues the gather
    # descriptor generation, and any dependent op in its in-order queue
    # stalls the whole gather stream (measured +80us).
    chunks = [tuple(ch) for ch in chunks]



    # per-dest slot base/K for filling
    dest_base = np.zeros(PT, dtype=np.int64)
    dest_K = np.ones(PT, dtype=np.int64)
    for doff, dc, cK, io, _pool in chunks:
        d = np.arange(dc)
        dest_base[doff : doff + dc] = io + d * cK
        dest_K[doff : doff + dc] = cK

    # slot array [cores, TOT] filled with ZROW, then scatter edge sources.
    src_trow = _trow(src_new, P)
    slots = np.full((n_cores, TOT), ZROW, dtype=np.int64)
    e_core = dst_new // P
    e_dloc = dst_new % P
    sort_k = np.argsort(e_core * n_nodes + e_dloc, kind="stable")
    sc, sd, ss = e_core[sort_k], e_dloc[sort_k], src_trow[sort_k]
    # rank within each (core,dest) run
    key = sc * n_nodes + sd
    first = np.r_[True, key[1:] != key[:-1]]
    run_start = np.maximum.accumulate(np.where(first, np.arange(key.size), 0))
    rank = np.arange(key.size) - run_start
    flat = dest_base[sd] + rank
    slots[sc, flat] = ss

    # wrap to [128, TOT//16] int16: element (p, s) = slots[s*16 + p%16]
    # (the gather ucode reads its own 16-partition block per Q7 core, so the
    # table must be replicated across all 128 partitions — a [16, W] tile
    # yields garbage gathers on hardware)
    assert TOT % 16 == 0
    wrapped = np.empty((n_cores, 128, TOT // 16), dtype=np.int16)
    for c in range(n_cores):
        w16 = slots[c].reshape(TOT // 16, 16).T.astype(np.int16)  # [16, TOT/16]
        wrapped[c] = np.tile(w16, (8, 1))

    return dict(
        P=P, PT=PT, NG=NG, TOT=TOT,
        Kg=[int(k) for k in Kg],
        chunks=chunks,
        new_id=new_id, orig_of_new=orig_of_new,
        deg_loc=deg_loc, gidx=wrapped,
    )


# -------------------- bass kernel builder --------------------
def _build_bass(n_nodes, n_cores, h, P, PT, NG, TOT, Kg, chunks,
                repeat=1, collective=True, LA_PIPE=2):
    import concourse.bass as bass
    import concourse.bacc as bacc
    import concourse.mybir as mybir
    import concourse.tile as tile
    from concourse import library_config

    dt = mybir.dt
    f32, bf16, i16 = dt.float32, dt.bfloat16, dt.int16
    AF = mybir.ActivationFunctionType
    NT = PT // 128  # node tiles per core
    KC = h // 128  # contraction chunks (2)
    G_LAST_REAL = P - (NG - 1) * 128  # real dests in the last group (68)

    nc = bacc.Bacc(dynamic_dma_scratch_size=49152)
    idx_in = nc.declare_dram_parameter("gidx", [128, TOT // 16], i16, isOutput=False)
    t0_in = nc.declare_dram_parameter("t0", [PT, h], bf16, isOutput=False)
    table0_in = nc.declare_dram_parameter("table0", [NTAB, h], bf16, isOutput=False)
    W_in = [nc.declare_dram_parameter(nm, [128, KC, h], bf16, isOutput=False)
            for nm in ("W1", "W1_1", "W2", "W3")]
    b_in = [nc.declare_dram_parameter(nm, [h], f32, isOutput=False)
            for nm in ("b1", "b1_1", "b2", "b3")]
    # staged structural constants (functions of the graph only)
    dinv_in = nc.declare_dram_parameter("dinv_nm", [128, NT], f32, isOutput=False)
    dinv2_in = nc.declare_dram_parameter("dinv2_nm", [128, NT], f32, isOutput=False)
    sqd_in = nc.declare_dram_parameter("sqd_row", [PT], f32, isOutput=False)
    ident_in = nc.declare_dram_parameter("ident", [128, 128], bf16, isOutput=False)
    out23_ext = nc.declare_dram_parameter("out23", [2, P, h], bf16, isOutput=True)

    with tile.TileContext(nc) as tc:
        with (
            tc.tile_pool(name="dram", bufs=1, space="DRAM") as dpool,
            tc.tile_pool(name="const", bufs=1) as cpool,
            tc.tile_pool(name="gather", bufs=7) as gpool,
            tc.tile_pool(name="rbuf", bufs=6) as rpool,
            tc.tile_pool(name="tt", bufs=2) as tpool,
            tc.tile_pool(name="xbuf", bufs=1) as xpool,
            tc.tile_pool(name="work", bufs=4) as wpool,
            tc.tile_pool(name="outs", bufs=12) as opool,
            tc.tile_pool(name="psum", bufs=6, space="PSUM") as ppool,
            tc.tile_pool(name="ptr", bufs=2, space="PSUM") as trpool,
        ):
            # ---- internal DRAM ---- (per-repeat for benchmark variants:
            # Tile requires a single writer for Shared DRAM)
            # AllGather staging for tables 1 and 2
            ag_r = [
                {t: dpool.tile([RR, h], bf16, name=f"ag{t}_{r}")
                 for t in (1, 2)}
                for r in range(repeat)
            ]
            if collective:
                tables_r = [
                    {t: dpool.tile([NTAB, h], bf16, addr_space="Shared",
                                   name=f"table{t}_{r}") for t in (1, 2)}
                    for r in range(repeat)
                ]
            else:  # timing-study variant: tables fed as plain inputs, no AG
                tin = {
                    t: nc.declare_dram_parameter(f"tbl{t}", [NTAB, h], bf16,
                                                 isOutput=False)
                    for t in (1, 2)
                }
                tables_r = [tin for _ in range(repeat)]

            # ---- constants ----
            # gidx first: it gates the first gather's descriptor generation,
            # which is the whole critical-path start (layer-0's table is a
            # host-staged input, so nothing else blocks the first gather).
            # Split the load so the first chunk's slice lands ~1.5us earlier.
            gidx = cpool.tile([128, TOT // 16], i16, name="gidx_sb")
            w0 = max(chunks[0][1] * chunks[0][2] // 16, 16)
            nc.sync.dma_start(gidx[:, :w0], idx_in[:, :w0])
            nc.sync.dma_start(gidx[:, w0:], idx_in[:, w0:])

            ident = cpool.tile([128, 128], bf16, name="ident")
            nc.sync.dma_start(ident[:], ident_in[:])
            dinv_nm = cpool.tile([128, NT], f32, name="dinv_nm")
            nc.sync.dma_start(dinv_nm[:], dinv_in[:])
            dinv2_nm = cpool.tile([128, NT], f32, name="dinv2_nm")
            nc.sync.dma_start(dinv2_nm[:], dinv2_in[:])
            sqd_row = cpool.tile([1, PT], f32, name="sqd_row")
            nc.sync.dma_start(sqd_row[:], sqd_in[None, :])

            # layer-0 self-loop tiles (bf16(dinv*x), host-staged)
            xall0 = xpool.tile([128, NT, h], bf16, tag="xall", name="xall0")
            nc.sync.dma_start(
                xall0[:], t0_in.rearrange("(t p) j -> p t j", p=128)
            )

            # needed once the first matmul runs (~10us in): these loads hide
            # under the layer-1 gathers
            w_sb = []
            for i in range(4):
                wb = cpool.tile([128, KC, h], bf16, name=f"wb{i}")
                nc.sync.dma_start(wb[:], W_in[i][:])
                w_sb.append(wb)
            b_sb = []
            for i in range(4):
                bt = cpool.tile([1, h], f32, name=f"bv{i}")
                nc.sync.dma_start(bt[:], b_in[i][None, :])
                b_sb.append(bt)

            rg = [list(range(n_cores))]
            # zero-fill the table pad area: the last group's missing dest
            # rows plus the 16 ZROW rows every gather's pad slots hit
            zpad = cpool.tile([128, h], bf16, name="zpad")
            nc.vector.memset(zpad[:], 0.0)
            ZB0 = P  # zpad rows directly follow the real dest rows
            for r in range(repeat):
                for t in (1, 2):
                    nc.sync.dma_start(
                        ag_r[r][t][ZB0:RR, :], zpad[: RR - ZB0, :]
                    )

            # chunks grouped by 128-dest tile
            by_group = [[] for _ in range(NG)]
            for ch in chunks:
                by_group[ch[0] // 128].append(ch)

            def mm_early(ps, ttT, t, wi):
                """Gather-independent part of ps = (Rb+ttT)^T @ W + sqd*b:
                self-loop rows and bias, accumulated on the PE ahead of the
                gather-dependent Rb matmuls."""
                for c in range(KC):
                    nc.tensor.matmul(
                        ps[:],
                        lhsT=ttT[:, c, :],
                        rhs=w_sb[wi][:, c, :],
                        start=(c == 0),
                        stop=False,
                    )
                nc.tensor.matmul(
                    ps[:],
                    lhsT=sqd_row[0:1, t * 128 : (t + 1) * 128],
                    rhs=b_sb[wi][:],
                    start=False,
                    stop=False,
                )

            def mm_rb(ps, Rb, wi):
                for c in range(KC):
                    nc.tensor.matmul(
                        ps[:],
                        lhsT=Rb[:, c, :],
                        rhs=w_sb[wi][:, c, :],
                        start=False,
                        stop=(c == KC - 1),
                    )

            def emit_ag(rep, t):
                """AllGather table t (t in {1,2})."""
                if not collective:
                    return
                nc.gpsimd.collective_compute(
                    "AllGather",
                    mybir.AluOpType.bypass,
                    replica_groups=rg,
                    ins=[ag_r[rep][t].opt()],
                    outs=[tables_r[rep][t].opt()],
                )

            def process_layer(rep, L, tt_tiles):
                """Per 128-dest group: gather in-edge rows from table L,
                fold/halve-reduce on DVE, add the self-loop rows (PE transpose
                of the resident tt tile), matmul + fused epilogue, emit either
                the next layer's AG staging tiles (L<2, with the region-A
                collective fired mid-layer) or the two output heads.

                Returns the next layer's tt tiles (or None for L=2)."""
                table = table0_in if L == 0 else tables_r[rep][L]
                next_tt = [None] * NG

                def finalize(g, Rb, ttT):
                    """Gather-dependent epilogue for group g: Rb matmuls,
                    activation, store. Runs LA groups behind the gather loop
                    so the in-order PE never parks a blocked Rb matmul in
                    front of later groups' independent work."""
                    rows = min(128, P - g * 128)
                    if L < 2:
                        ps = ps_of[g][0]
                        mm_rb(ps, Rb, L)
                        tt = tpool.tile([128, h], bf16, tag=f"tt{g}",
                                        name=f"ttl{rep}_{L}_{g}")
                        nc.scalar.activation(
                            tt[:], ps[:], AF.Relu, scale=dinv2_nm[:, g : g + 1]
                        )
                        nc.sync.dma_start(
                            ag_r[rep][L + 1][g * 128 : g * 128 + rows, :],
                            tt[:rows, :],
                        )
                        next_tt[g] = tt
                    elif batch and g in batch:
                        # tail trim: the two last FULL groups and both heads
                        # share ONE store instead of dripping four through
                        # the 625ns/copy HWDGE at the drain
                        bi = g - batch[0]
                        ps2, ps3 = ps_of[g]
                        mm_rb(ps2, Rb, 2)
                        nc.scalar.activation(
                            b23[:, bi, 0, :], ps2[:], AF.Copy,
                            scale=dinv_nm[:, g : g + 1],
                        )
                        mm_rb(ps3, Rb, 3)
                        nc.scalar.activation(
                            b23[:, bi, 1, :], ps3[:], AF.Copy,
                            scale=dinv_nm[:, g : g + 1],
                        )
                        if g == batch[1]:
                            r0 = batch[0] * 128
                            for c in range(2):
                                obv = out23_ext[c, r0 : r0 + 256, :].rearrange(
                                    "(t p) j -> p t j", p=128
                                )
                                nc.sync.dma_start(obv, b23[:, :, c, :])
                    else:
                        ps2, ps3 = ps_of[g]
                        o23 = opool.tile([128, 2, h], bf16, tag="hsb",
                                         name=f"o23_{rep}_{g}")
                        mm_rb(ps2, Rb, 2)
                        nc.scalar.activation(
                            o23[:, 0, :], ps2[:], AF.Copy,
                            scale=dinv_nm[:, g : g + 1],
                        )
                        mm_rb(ps3, Rb, 3)
                        nc.scalar.activation(
                            o23[:, 1, :], ps3[:], AF.Copy,
                            scale=dinv_nm[:, g : g + 1],
                        )
                        nc.sync.dma_start(
                            out23_ext[:, g * 128 : g * 128 + rows, :].rearrange(
                                "c r j -> r c j"
                            ),
                            o23[:rows, :, :],
                        )

                # identity order: region A's tiles are finalized (and its
                # collective fired) before region B's; the degree-sorted
                # layout already puts the big-K groups first and drains
                # through the small last group.
                order = list(range(NG))
                LA = LA_PIPE
                ps_of = {}
                pending = []
                batch = None
                b23 = None
                if L == 2 and NG >= 3:
                    cand = (order[-3], order[-2])
                    if (cand[1] == cand[0] + 1
                            and (cand[0] + 2) * 128 <= P):
                        batch = cand
                        b23 = xpool.tile([128, 2, 2, h], bf16, tag="b23",
                                         name=f"b23_{rep}")

                def run_finalize(entry):
                    fg, fRb, fttT = entry
                    finalize(fg, fRb, fttT)

                for g in order:
                    Rb = rpool.tile([128, KC, 128], bf16, tag="Rbg",
                                    name=f"Rb{rep}_{L}_{g}")
                    # all-pad dest columns have no gather chunk: zero them so
                    # the matmul lhsT is fully initialized
                    covered = np.zeros(128, dtype=bool)
                    for doff, dc, K, ioff, on_pool in by_group[g]:
                        covered[doff % 128 : doff % 128 + dc] = True
                    a = 0
                    while a < 128:
                        if covered[a]:
                            a += 1
                            continue
                        b = a
                        while b < 128 and not covered[b]:
                            b += 1
                        nc.vector.memset(Rb[:, :, a:b], 0.0)
                        a = b
                    for ci, (doff, dc, K, ioff, on_pool) in enumerate(by_group[g]):
                        n_idx = dc * K
                        gt = gpool.tile([128, KC, n_idx], bf16, tag="gt",
                                        name=f"gt{rep}_{L}_{g}_{ci}")
                        nc.gpsimd.dma_gather(
                            gt[:],
                            table[:, :],
                            gidx[:, ioff // 16 : (ioff + n_idx) // 16],
                            n_idx,
                            n_idx,
                            h,
                            transpose=True,
                            single_packet=(n_idx <= 896),
                        )
                        # fold-odd-then-halve pair-add cascade (2x mode,
                        # in place)
                        cK = K
                        g4 = gt.rearrange("p c (d k) -> p c d k", k=K)
                        while cK > 3:
                            if cK % 2:
                                nc.vector.tensor_add(
                                    g4[:, :, :, 0],
                                    g4[:, :, :, 0],
                                    g4[:, :, :, cK - 1],
                                )
                                cK -= 1
                            nh = cK // 2
                            nc.vector.tensor_add(
                                g4[:, :, :, 0:nh],
                                g4[:, :, :, 0:nh],
                                g4[:, :, :, nh:cK],
                            )
                            cK = nh
                        # short tail straight to the bf16 matmul lhsT
                        # (engine accumulates wider; single rounding at the
                        # write, same as the old f32->bf16 copy path)
                        RbS = Rb[:, :, doff % 128 : doff % 128 + dc]
                        if cK == 1:
                            nc.vector.tensor_copy(RbS, g4[:, :, :, 0])
                        else:
                            # [p,c,dc]-shaped strided views; cheap adds beat
                            # a 1x reduce that rereads every slot
                            nc.vector.tensor_add(
                                RbS, g4[:, :, :, 0], g4[:, :, :, 1]
                            )
                            if cK == 3:
                                nc.vector.tensor_add(
                                    RbS, RbS, g4[:, :, :, 2]
                                )
                    # self-loop rows: transpose this group's tt tile on the PE
                    # (feature-major like Rb), stage to SBUF via the idle ACT
                    ptt = trpool.tile([128, KC, 128], bf16, tag="ptt",
                                      name=f"ptt{rep}_{L}_{g}")
                    for c in range(KC):
                        nc.tensor.transpose(
                            ptt[:, c, :],
                            tt_tiles[g][:, c * 128 : (c + 1) * 128],
                            ident[:],
                        )
                    ttT = rpool.tile([128, KC, 128], bf16, tag="ttT",
                                     name=f"ttT{rep}_{L}_{g}")
                    nc.scalar.copy(ttT[:], ptt[:])
                    if L < 2:
                        ps = ppool.tile([128, h], f32, tag="ps",
                                        name=f"ps{rep}_{L}_{g}")
                        mm_early(ps, ttT, g, L)
                        ps_of[g] = (ps,)
                    else:
                        ps2 = ppool.tile([128, h], f32, tag="ps",
                                         name=f"ps2_{rep}_{g}")
                        mm_early(ps2, ttT, g, 2)
                        ps3 = ppool.tile([128, h], f32, tag="ps",
                                         name=f"ps3_{rep}_{g}")
                        mm_early(ps3, ttT, g, 3)
                        ps_of[g] = (ps2, ps3)
                    pending.append((g, Rb, ttT))
                    if len(pending) > LA:
                        run_finalize(pending.pop(0))
                for entry in pending:
                    run_finalize(entry)
                # the next table's AllGather: exposed at the layer boundary
                if L < 2:
                    emit_ag(rep, L + 1)
                return next_tt

            for rep in range(repeat):
                if rep == 0:
                    xall = xall0
                else:
                    xall = xpool.tile([128, NT, h], bf16, tag="xall",
                                      name=f"xall{rep}")
                    nc.sync.dma_start(
                        xall[:], t0_in.rearrange("(t p) j -> p t j", p=128)
                    )
                tt_tiles = [xall[:, t, :] for t in range(NT)]
                for L in range(3):
                    tt_tiles = process_layer(rep, L, tt_tiles)

    nc.compile()
    return nc


# -------------------- public entry --------------------
def kernel(x, edge_index, W1, b1, W1_1, b1_1, W2, b2, W3, b3):
    import ml_dtypes
    from concourse.bass_utils import run_bass_kernel_spmd

    bf16 = ml_dtypes.bfloat16
    x = np.asarray(x, dtype=np.float32)
    edge_index = np.asarray(edge_index, dtype=np.int32)
    n_nodes, h = x.shape
    meta = _prep_graph(edge_index, n_nodes, C)
    P, PT, NG, TOT = meta["P"], meta["PT"], meta["NG"], meta["TOT"]

    key = (n_nodes, h, tuple(meta["Kg"]), TOT)
    if key not in _KERNEL_CACHE:
        _KERNEL_CACHE[key] = _build_bass(
            n_nodes, C, h, P, PT, NG, TOT, meta["Kg"], meta["chunks"],
        )
    nc = _KERNEL_CACHE[key]

    oon = meta["orig_of_new"]
    # staged structural constants
    deg_loc = meta["deg_loc"]  # [C, PT] f32, padded with 1.0
    sqd = np.sqrt(deg_loc)  # [C, PT]
    dinv = (1.0 / sqd).astype(np.float32)
    NT = PT // 128
    dinv_nm = np.ascontiguousarray(
        dinv.reshape(C, NT, 128).transpose(0, 2, 1), dtype=np.float32
    )  # [C, 128, NT]
    dinv2_nm = np.ascontiguousarray(dinv_nm * dinv_nm, dtype=np.float32)
    ident = np.eye(128, dtype=np.float32).astype(bf16)

    # layer-0 gather table: bf16(dinv * x) in the region layout, host-staged
    x_new = x[oon]  # [N, h], ordered by new id
    dinv_new = np.concatenate([dinv[c, :P] for c in range(C)])  # [N]
    scaled = (dinv_new[:, None] * x_new).astype(bf16)  # [N, h]
    table0 = np.zeros((NTAB, h), dtype=bf16)
    table0[_trow(np.arange(n_nodes), P)] = scaled

    Ws = {"W1": W1, "W1_1": W1_1, "W2": W2, "W3": W3}
    bs = {"b1": b1, "b1_1": b1_1, "b2": b2, "b3": b3}
    # weights pre-arranged to the PE lhs layout [(c p) j -> p c j] in bf16
    Wstage = {
        k: np.ascontiguousarray(
            np.asarray(v, dtype=np.float32)
            .reshape(2, 128, h)
            .transpose(1, 0, 2)
            .astype(bf16)
        )
        for k, v in Ws.items()
    }
    in_maps = []
    for c in range(C):
        t0 = np.zeros((PT, h), dtype=bf16)
        t0[:P] = scaled[c * P : (c + 1) * P]
        m = {
            "t0": t0,
            "table0": table0,
            "gidx": np.ascontiguousarray(meta["gidx"][c]),
            "dinv_nm": dinv_nm[c],
            "dinv2_nm": dinv2_nm[c],
            "sqd_row": np.ascontiguousarray(sqd[c], dtype=np.float32),
            "ident": ident,
        }
        for k, v in Wstage.items():
            m[k] = v
        for k, v in bs.items():
            m[k] = np.ascontiguousarray(v, dtype=np.float32)
        in_maps.append(m)

    global LAST_RESULTS
    LAST_RESULTS = run_bass_kernel_spmd(nc, in_maps, core_ids=list(range(C)))
    res = LAST_RESULTS.results

    out2_new = np.concatenate(
        [np.asarray(res[c]["out23"][0]) for c in range(C)], axis=0
    ).astype(np.float32)
    out3_new = np.concatenate(
        [np.asarray(res[c]["out23"][1]) for c in range(C)], axis=0
    ).astype(np.float32)
    new_id = meta["new_id"]
    return out2_new[new_id], out3_new[new_id]


# revision 21
# speedup vs baseline: 1.0008x; 1.0008x over previous
"""GCNEncoder Trainium2 kernel (8 NeuronCores, SPMD).

Strategy (graph/data parallel, per sharding hint):
  - Nodes are dealt round-robin-by-degree across 8 cores (2500 each); the
    [H,H] weights are replicated.
  - Layer-0's gather table (bf16(dinv * x)) is a function of the INPUT, so
    the host stages it directly into each core's HBM: no AllGather, no x
    load, no on-chip prologue for layer 1.
  - Per remaining GCN layer: each core's dinv-scaled bf16 output tiles are
    AllGathered into every core's HBM (one collective per table; only two
    AllGathers total are exposed at the layer boundaries).
  - Message aggregation = segment-sum over in-edges:  per 128-destination
    group, a transposed dma_gather pulls the source rows (feature-major:
    [128h, 2, 128*K]) and a fold-then-halve DVE cascade sums each
    destination's K slots (padding slots point at an all-zero table row).
  - Self-loop contributions never touch the gather: each destination's own
    row already sits in SBUF (the tt tile the core just produced); a PE
    transpose (identity matmul) + ACT copy stages it as an extra matmul
    accumulated into the same PSUM tile as the gathered aggregate.
  - The GCNConv reorder agg(x) @ W == agg(x @ W) lets one aggregation per
    layer feed the [HxH] matmul afterwards; out2/out3 share the layer-3
    aggregation.  norm = dinv[row]*dinv[col] factorizes into the table
    pre-scale and a per-destination post-scale fused into the PSUM->SBUF
    activation (bias is added via a K=1 rank-1 matmul of sqrt(deg) x b).
  - Gather chunk boundaries come from a per-128-group DP that trades slot
    padding (1.42 ns/slot of serial gather DMA) against per-chunk fixed
    cost; structural constants (dinv, sqrt(deg), bf16 weights, identity)
    are staged from the host.

Self-contained: hardcodes the problem shapes; only needs numpy + concourse.
"""

import os

import numpy as np

# Defensive: a previous process dying mid-run can leave /dev/neuron* wedged
# (NRT_EXEC_UNIT_UNRECOVERABLE on the next open). Asking NRT to reset cores
# at init recovers it; must be set before the first jax/PJRT init.
os.environ.setdefault("NEURON_RT_RESET_CORES", "1")

# -------------------- problem constants --------------------
N_NODES = 20000
N_EDGES = 320000
H = 256
C = 8  # cores

MAXI = 2944  # max gather indices per dma_gather (descriptor-ring limit)
DP_LAM = 100.0
DP_GAMMA = 0.3

# table layout: per-rank stripe of PT dest rows + 16 zero-pad rows
P_CONST = N_NODES // C          # 2500
PT_CONST = ((P_CONST + 127) // 128) * 128  # 2560
NG_CONST = PT_CONST // 128      # 20
RR = P_CONST + 16               # 2516 rows per rank (incl zpad)
NTAB = C * RR                   # 20128 table rows
ZROW = P_CONST                  # 2500: rank0's first zero-pad row

_KERNEL_CACHE = {}
LAST_RESULTS = None  # BassKernelResults of the most recent run (for profiling)


def _trow(n, P):
    """Table row of new node id n (vectorized)."""
    n = np.asarray(n)
    return (n // P) * RR + (n % P)


def _dve_cost(K):
    """DVE ns per destination for a K-slot chunk: fold-odd-then-halve
    cascade (2x mode) + a short strided-add tail into the matmul lhsT."""
    if K == 1:
        return 1.04
    c, cost = K, 0.0
    while c > 3:
        if c % 2:
            cost += 1.04  # fold the odd slot into slot 0
            c -= 1
        cost += 1.04 * (c // 2)
        c //= 2
    return cost + max(2.08 * (c - 1), 2.08)


# -------------------- host-side graph prep --------------------
def _prep_graph(edge_index, n_nodes, n_cores):
    """Partition nodes, build per-core padded gather-slot index arrays.

    Returns dict with permutation, per-core degree arrays, gather indices.
    """
    P = n_nodes // n_cores  # nodes per core
    row = edge_index[0].astype(np.int64)
    col = edge_index[1].astype(np.int64)
    deg = np.bincount(col, minlength=n_nodes).astype(np.int64) + 1  # incl loop

    # deal nodes round-robin by ascending degree -> every core gets an
    # almost identical degree profile, sorted ascending within the core.
    order = np.argsort(deg, kind="stable")
    pos = np.empty(n_nodes, dtype=np.int64)
    pos[order] = np.arange(n_nodes)
    # descending degree within the core: the zero-pad dests (local ids >= P)
    # then share a gather chunk with the LOWEST-degree nodes, so their K (and
    # wasted slots) is small instead of the global max
    new_id = (pos % n_cores) * P + (P - 1 - pos // n_cores)  # old -> new
    orig_of_new = np.empty(n_nodes, dtype=np.int64)
    orig_of_new[new_id] = np.arange(n_nodes)

    # self-loops are folded in on-chip; only real edges gather
    src_new = new_id[row]
    dst_new = new_id[col]

    PT = ((P + 127) // 128) * 128  # padded dest count per core
    NG = PT // 128  # 128-dest groups

    deg_new = deg[orig_of_new]  # per new id (includes the +1 self loop)
    k_new = deg_new - 1  # gather slots actually needed per dest

    # per-core padded arrays
    deg_loc = np.ones((n_cores, PT), dtype=np.float32)
    k_loc = np.zeros((n_cores, PT), dtype=np.int64)
    for c in range(n_cores):
        deg_loc[c, :P] = deg_new[c * P : (c + 1) * P]
        k_loc[c, :P] = k_new[c * P : (c + 1) * P]

    # max (over cores) slot count within a local-dest range (raw, unrounded)
    def range_K(lo, hi):
        m = 0
        for c in range(n_cores):
            seg = k_loc[c, lo:hi]
            if seg.size:
                m = max(m, int(seg.max()))
        return m

    # Chunking: per 128-dest group, DP over split points (32-dest
    # granularity keeps n_idx%128 with K%4) minimizing joint serial cost:
    #   1.42*slots (gather DMA) + 0.7*dve_cost (co-critical DVE reduce)
    #   + LAM_NS per chunk,   subject to dc*K <= MAXI.
    # The chunk may also round K UP past the minimum when a larger K has a
    # cheaper DVE-reduce tail.
    LAM_NS = DP_LAM  # per-chunk fixed cost
    GAMMA = DP_GAMMA  # DVE-reduce weight vs gather-DMA in the joint chunk cost

    def seg_cost(dc, mink):
        # K granularity follows the n_idx%128 transpose-gather constraint
        if mink == 0:
            # all-pad segment: no gather at all — the kernel memsets the
            # matmul-lhsT columns instead (sentinel K=0)
            return (0.0, 0)
        # dc*K must be %128: with dc = 16*m, K must be %(8//gcd(m,8))
        import math as _math
        step = 8 // _math.gcd(dc // 16, 8)
        base = ((mink + step - 1) // step) * step
        best = None
        for cK in range(base, base + 4 * step, step):
            if dc * cK > MAXI:
                continue
            c = dc * (1.42 * cK + GAMMA * _dve_cost(cK))
            if best is None or c < best[0]:
                best = (c, cK)
        return best

    chunks = []  # (dest_off_in_core, dc, K, idx_off)
    Kg = []
    ioff = 0
    STEP = 16
    for g in range(NG):
        base = g * 128
        nseg = 128 // STEP
        INF = float("inf")
        dp = [INF] * (nseg + 1)
        dp[0] = 0.0
        prev = [(0, 4)] * (nseg + 1)
        for i in range(1, nseg + 1):
            for j in range(i):
                dc = (i - j) * STEP
                mink = range_K(base + j * STEP, base + i * STEP)
                sc = seg_cost(dc, mink)
                if sc is None:
                    continue
                cost = dp[j] + sc[0] + LAM_NS
                if cost < dp[i]:
                    dp[i] = cost
                    prev[i] = (j, sc[1])
        # walk back
        cuts = []
        i = nseg
        while i > 0:
            j, cK = prev[i]
            cuts.append((j, i, cK))
            i = j
        gK = 4
        for j, i, cK in reversed(cuts):
            dc = (i - j) * STEP
            if cK == 0:
                continue  # all-pad: kernel memsets these Rb columns
            assert (dc * cK) % 128 == 0 and dc * cK <= MAXI
            chunks.append([base + j * STEP, dc, cK, ioff, 0])
            ioff += dc * cK
            gK = max(gK, cK)
        Kg.append(gK)
    TOT = int(ioff)  # slots per core (same for all cores)

    # NOTE: tensor work must stay off the Pool engine — it issues the gather
    # descriptor generation, and any dependent op in its in-order queue
    # stalls the whole gather stream (measured +80us).
    chunks = [tuple(ch) for ch in chunks]



    # per-dest slot base/K for filling
    dest_base = np.zeros(PT, dtype=np.int64)
    dest_K = np.ones(PT, dtype=np.int64)
    for doff, dc, cK, io, _pool in chunks:
        d = np.arange(dc)
        dest_base[doff : doff + dc] = io + d * cK
        dest_K[doff : doff + dc] = cK

    # slot array [cores, TOT] filled with ZROW, then scatter edge sources.
    src_trow = _trow(src_new, P)
    slots = np.full((n_cores, TOT), ZROW, dtype=np.int64)
    e_core = dst_new // P
    e_dloc = dst_new % P
    sort_k = np.argsort(e_core * n_nodes + e_dloc, kind="stable")
    sc, sd, ss = e_core[sort_k], e_dloc[sort_k], src_trow[sort_k]
    # rank within each (core,dest) run
    key = sc * n_nodes + sd
    first = np.r_[True, key[1:] != key[:-1]]
    run_start = np.maximum.accumulate(np.where(first, np.arange(key.size), 0))
    rank = np.arange(key.size) - run_start
    flat = dest_base[sd] + rank
    slots[sc, flat] = ss

    # wrap to [128, TOT//16] int16: element (p, s) = slots[s*16 + p%16]
    # (the gather ucode reads its own 16-partition block per Q7 core, so the
    # table must be replicated across all 128 partitions — a [16, W] tile
    # yields garbage gathers on hardware)
    assert TOT % 16 == 0
    wrapped = np.empty((n_cores, 128, TOT // 16), dtype=np.int16)
    for c in range(n_cores):
        w16 = slots[c].reshape(TOT // 16, 16).T.astype(np.int16)  # [16, TOT/16]
        wrapped[c] = np.tile(w16, (8, 1))

    return dict(
        P=P, PT=PT, NG=NG, TOT=TOT,
        Kg=[int(k) for k in Kg],
        chunks=chunks,
        new_id=new_id, orig_of_new=orig_of_new,
        deg_loc=deg_loc, gidx=wrapped,
    )


# -------------------- bass kernel builder --------------------
def _build_bass(n_nodes, n_cores, h, P, PT, NG, TOT, Kg, chunks,
                repeat=1, collective=True, LA_PIPE=2):
    import concourse.bass as bass
    import concourse.bacc as bacc
    import concourse.mybir as mybir
    import concourse.tile as tile
    from concourse import library_config

    dt = mybir.dt
    f32, bf16, i16 = dt.float32, dt.bfloat16, dt.int16
    AF = mybir.ActivationFunctionType
    NT = PT // 128  # node tiles per core
    KC = h // 128  # contraction chunks (2)
    G_LAST_REAL = P - (NG - 1) * 128  # real dests in the last group (68)

    nc = bacc.Bacc(dynamic_dma_scratch_size=49152)
    idx_in = nc.declare_dram_parameter("gidx", [128, TOT // 16], i16, isOutput=False)
    t0_in = nc.declare_dram_parameter("t0", [PT, h], bf16, isOutput=False)
    table0_in = nc.declare_dram_parameter("table0", [NTAB, h], bf16, isOutput=False)
    W_in = [nc.declare_dram_parameter(nm, [128, KC, h], bf16, isOutput=False)
            for nm in ("W1", "W1_1", "W2", "W3")]
    b_in = [nc.declare_dram_parameter(nm, [h], f32, isOutput=False)
            for nm in ("b1", "b1_1", "b2", "b3")]
    # staged structural constants (functions of the graph only)
    dinv_in = nc.declare_dram_parameter("dinv_nm", [128, NT], f32, isOutput=False)
    dinv2_in = nc.declare_dram_parameter("dinv2_nm", [128, NT], f32, isOutput=False)
    sqd_in = nc.declare_dram_parameter("sqd_row", [PT], f32, isOutput=False)
    ident_in = nc.declare_dram_parameter("ident", [128, 128], bf16, isOutput=False)
    out23_ext = nc.declare_dram_parameter("out23", [2, P, h], bf16, isOutput=True)

    with tile.TileContext(nc) as tc:
        with (
            tc.tile_pool(name="dram", bufs=1, space="DRAM") as dpool,
            tc.tile_pool(name="const", bufs=1) as cpool,
            tc.tile_pool(name="gather", bufs=7) as gpool,
            tc.tile_pool(name="rbuf", bufs=6) as rpool,
            tc.tile_pool(name="tt", bufs=2) as tpool,
            tc.tile_pool(name="xbuf", bufs=1) as xpool,
            tc.tile_pool(name="work", bufs=4) as wpool,
            tc.tile_pool(name="outs", bufs=12) as opool,
            tc.tile_pool(name="psum", bufs=6, space="PSUM") as ppool,
            tc.tile_pool(name="ptr", bufs=2, space="PSUM") as trpool,
        ):
            # ---- internal DRAM ---- (per-repeat for benchmark variants:
            # Tile requires a single writer for Shared DRAM)
            # AllGather staging for tables 1 and 2
            ag_r = [
                {t: dpool.tile([RR, h], bf16, name=f"ag{t}_{r}")
                 for t in (1, 2)}
                for r in range(repeat)
            ]
            if collective:
                tables_r = [
                    {t: dpool.tile([NTAB, h], bf16, addr_space="Shared",
                                   name=f"table{t}_{r}") for t in (1, 2)}
                    for r in range(repeat)
                ]
            else:  # timing-study variant: tables fed as plain inputs, no AG
                tin = {
                    t: nc.declare_dram_parameter(f"tbl{t}", [NTAB, h], bf16,
                                                 isOutput=False)
                    for t in (1, 2)
                }
                tables_r = [tin for _ in range(repeat)]

            # ---- constants ----
            # gidx first: it gates the first gather's descriptor generation,
            # which is the whole critical-path start (layer-0's table is a
            # host-staged input, so nothing else blocks the first gather).
            # Split the load so the first chunk's slice lands ~1.5us earlier.
            gidx = cpool.tile([128, TOT // 16], i16, name="gidx_sb")
            w0 = max(chunks[0][1] * chunks[0][2] // 16, 16)
            nc.sync.dma_start(gidx[:, :w0], idx_in[:, :w0])
            nc.sync.dma_start(gidx[:, w0:], idx_in[:, w0:])

            ident = cpool.tile([128, 128], bf16, name="ident")
            nc.sync.dma_start(ident[:], ident_in[:])
            dinv_nm = cpool.tile([128, NT], f32, name="dinv_nm")
            nc.sync.dma_start(dinv_nm[:], dinv_in[:])
            dinv2_nm = cpool.tile([128, NT], f32, name="dinv2_nm")
            nc.sync.dma_start(dinv2_nm[:], dinv2_in[:])
            sqd_row = cpool.tile([1, PT], f32, name="sqd_row")
            nc.sync.dma_start(sqd_row[:], sqd_in[None, :])

            # layer-0 self-loop tiles (bf16(dinv*x), host-staged)
            xall0 = xpool.tile([128, NT, h], bf16, tag="xall", name="xall0")
            nc.sync.dma_start(
                xall0[:], t0_in.rearrange("(t p) j -> p t j", p=128)
            )

            # needed once the first matmul runs (~10us in): these loads hide
            # under the layer-1 gathers
            w_sb = []
            for i in range(4):
                wb = cpool.tile([128, KC, h], bf16, name=f"wb{i}")
                nc.sync.dma_start(wb[:], W_in[i][:])
                w_sb.append(wb)
            b_sb = []
            for i in range(4):
                bt = cpool.tile([1, h], f32, name=f"bv{i}")
                nc.sync.dma_start(bt[:], b_in[i][None, :])
                b_sb.append(bt)

            rg = [list(range(n_cores))]
            # zero-fill the table pad area: the last group's missing dest
            # rows plus the 16 ZROW rows every gather's pad slots hit
            zpad = cpool.tile([128, h], bf16, name="zpad")
            nc.vector.memset(zpad[:], 0.0)
            ZB0 = P  # zpad rows directly follow the real dest rows
            for r in range(repeat):
                for t in (1, 2):
                    nc.sync.dma_start(
                        ag_r[r][t][ZB0:RR, :], zpad[: RR - ZB0, :]
                    )

            # chunks grouped by 128-dest tile
            by_group = [[] for _ in range(NG)]
            for ch in chunks:
                by_group[ch[0] // 128].append(ch)

            def mm_early(ps, ttT, t, wi):
                """Gather-independent part of ps = (Rb+ttT)^T @ W + sqd*b:
                self-loop rows and bias, accumulated on the PE ahead of the
                gather-dependent Rb matmuls."""
                for c in range(KC):
                    nc.tensor.matmul(
                        ps[:],
                        lhsT=ttT[:, c, :],
                        rhs=w_sb[wi][:, c, :],
                        start=(c == 0),
                        stop=False,
                    )
                nc.tensor.matmul(
                    ps[:],
                    lhsT=sqd_row[0:1, t * 128 : (t + 1) * 128],
                    rhs=b_sb[wi][:],
                    start=False,
                    stop=False,
                )

            def mm_rb(ps, Rb, wi):
                for c in range(KC):
                    nc.tensor.matmul(
                        ps[:],
                        lhsT=Rb[:, c, :],
                        rhs=w_sb[wi][:, c, :],
                        start=False,
                        stop=(c == KC - 1),
                    )

            def emit_ag(rep, t):
                """AllGather table t (t in {1,2})."""
                if not collective:
                    return
                nc.gpsimd.collective_compute(
                    "AllGather",
                    mybir.AluOpType.bypass,
                    replica_groups=rg,
                    ins=[ag_r[rep][t].opt()],
                    outs=[tables_r[rep][t].opt()],
                )

            def process_layer(rep, L, tt_tiles):
                """Per 128-dest group: gather in-edge rows from table L,
                fold/halve-reduce on DVE, add the self-loop rows (PE transpose
                of the resident tt tile), matmul + fused epilogue, emit either
                the next layer's AG staging tiles (L<2, with the region-A
                collective fired mid-layer) or the two output heads.

                Returns the next layer's tt tiles (or None for L=2)."""
                table = table0_in if L == 0 else tables_r[rep][L]
                next_tt = [None] * NG

                def finalize(g, Rb, ttT):
                    """Gather-dependent epilogue for group g: Rb matmuls,
                    activation, store. Runs LA groups behind the gather loop
                    so the in-order PE never parks a blocked Rb matmul in
                    front of later groups' independent work."""
                    rows = min(128, P - g * 128)
                    if L < 2:
                        ps = ps_of[g][0]
                        mm_rb(ps, Rb, L)
                        tt = tpool.tile([128, h], bf16, tag=f"tt{g}",
                                        name=f"ttl{rep}_{L}_{g}")
                        nc.scalar.activation(
                            tt[:], ps[:], AF.Relu, scale=dinv2_nm[:, g : g + 1]
                        )
                        nc.sync.dma_start(
                            ag_r[rep][L + 1][g * 128 : g * 128 + rows, :],
                            tt[:rows, :],
                        )
                        next_tt[g] = tt
                    elif batch and g in batch:
                        # tail trim: the two last FULL groups and both heads
                        # share ONE store instead of dripping four through
                        # the 625ns/copy HWDGE at the drain
                        bi = g - batch[0]
                        ps2, ps3 = ps_of[g]
                        mm_rb(ps2, Rb, 2)
                        nc.scalar.activation(
                            b23[:, bi, 0, :], ps2[:], AF.Copy,
                            scale=dinv_nm[:, g : g + 1],
                        )
                        mm_rb(ps3, Rb, 3)
                        nc.scalar.activation(
                            b23[:, bi, 1, :], ps3[:], AF.Copy,
                            scale=dinv_nm[:, g : g + 1],
                        )
                        if g == batch[1]:
                            r0 = batch[0] * 128
                            for c in range(2):
                                obv = out23_ext[c, r0 : r0 + 256, :].rearrange(
                                    "(t p) j -> p t j", p=128
                                )
                                nc.sync.dma_start(obv, b23[:, :, c, :])
                    else:
                        ps2, ps3 = ps_of[g]
                        o23 = opool.tile([128, 2, h], bf16, tag="hsb",
                                         name=f"o23_{rep}_{g}")
                        mm_rb(ps2, Rb, 2)
                        nc.scalar.activation(
                            o23[:, 0, :], ps2[:], AF.Copy,
                            scale=dinv_nm[:, g : g + 1],
                        )
                        mm_rb(ps3, Rb, 3)
                        nc.scalar.activation(
                            o23[:, 1, :], ps3[:], AF.Copy,
                            scale=dinv_nm[:, g : g + 1],
                        )
                        nc.sync.dma_start(
                            out23_ext[:, g * 128 : g * 128 + rows, :].rearrange(
                                "c r j -> r c j"
                            ),
                            o23[:rows, :, :],
                        )

                # identity order: region A's tiles are finalized (and its
                # collective fired) before region B's; the degree-sorted
                # layout already puts the big-K groups first and drains
                # through the small last group.
                order = list(range(NG))
                LA = LA_PIPE
                ps_of = {}
                pending = []
                batch = None
                b23 = None
                if L == 2 and NG >= 3:
                    # batch the THIRD- and SECOND-to-last groups: their big
                    # combined store then overlaps the last two groups'
                    # compute instead of sitting at the drain
                    cand = (order[-4], order[-3])
                    if (cand[1] == cand[0] + 1
                            and (cand[0] + 2) * 128 <= P):
                        batch = cand
                        b23 = xpool.tile([128, 2, 2, h], bf16, tag="b23",
                                         name=f"b23_{rep}")

                def run_finalize(entry):
                    fg, fRb, fttT = entry
                    finalize(fg, fRb, fttT)

                for g in order:
                    Rb = rpool.tile([128, KC, 128], bf16, tag="Rbg",
                                    name=f"Rb{rep}_{L}_{g}")
                    # all-pad dest columns have no gather chunk: zero them so
                    # the matmul lhsT is fully initialized
                    covered = np.zeros(128, dtype=bool)
                    for doff, dc, K, ioff, on_pool in by_group[g]:
                        covered[doff % 128 : doff % 128 + dc] = True
                    a = 0
                    while a < 128:
                        if covered[a]:
                            a += 1
                            continue
                        b = a
                        while b < 128 and not covered[b]:
                            b += 1
                        nc.vector.memset(Rb[:, :, a:b], 0.0)
                        a = b
                    for ci, (doff, dc, K, ioff, on_pool) in enumerate(by_group[g]):
                        n_idx = dc * K
                        gt = gpool.tile([128, KC, n_idx], bf16, tag="gt",
                                        name=f"gt{rep}_{L}_{g}_{ci}")
                        nc.gpsimd.dma_gather(
                            gt[:],
                            table[:, :],
                            gidx[:, ioff // 16 : (ioff + n_idx) // 16],
                            n_idx,
                            n_idx,
                            h,
                            transpose=True,
                            single_packet=(n_idx <= 896),
                        )
                        # fold-odd-then-halve pair-add cascade (2x mode,
                        # in place)
                        cK = K
                        g4 = gt.rearrange("p c (d k) -> p c d k", k=K)
                        while cK > 3:
                            if cK % 2:
                                nc.vector.tensor_add(
                                    g4[:, :, :, 0],
                                    g4[:, :, :, 0],
                                    g4[:, :, :, cK - 1],
                                )
                                cK -= 1
                            nh = cK // 2
                            nc.vector.tensor_add(
                                g4[:, :, :, 0:nh],
                                g4[:, :, :, 0:nh],
                                g4[:, :, :, nh:cK],
                            )
                            cK = nh
                        # short tail straight to the bf16 matmul lhsT
                        # (engine accumulates wider; single rounding at the
                        # write, same as the old f32->bf16 copy path)
                        RbS = Rb[:, :, doff % 128 : doff % 128 + dc]
                        if cK == 1:
                            nc.vector.tensor_copy(RbS, g4[:, :, :, 0])
                        else:
                            # [p,c,dc]-shaped strided views; cheap adds beat
                            # a 1x reduce that rereads every slot
                            nc.vector.tensor_add(
                                RbS, g4[:, :, :, 0], g4[:, :, :, 1]
                            )
                            if cK == 3:
                                nc.vector.tensor_add(
                                    RbS, RbS, g4[:, :, :, 2]
                                )
                    # self-loop rows: transpose this group's tt tile on the PE
                    # (feature-major like Rb), stage to SBUF via the idle ACT
                    ptt = trpool.tile([128, KC, 128], bf16, tag="ptt",
                                      name=f"ptt{rep}_{L}_{g}")
                    for c in range(KC):
                        nc.tensor.transpose(
                            ptt[:, c, :],
                            tt_tiles[g][:, c * 128 : (c + 1) * 128],
                            ident[:],
                        )
                    ttT = rpool.tile([128, KC, 128], bf16, tag="ttT",
                                     name=f"ttT{rep}_{L}_{g}")
                    nc.scalar.copy(ttT[:], ptt[:])
                    if L < 2:
                        ps = ppool.tile([128, h], f32, tag="ps",
                                        name=f"ps{rep}_{L}_{g}")
                        mm_early(ps, ttT, g, L)
                        ps_of[g] = (ps,)
                    else:
                        ps2 = ppool.tile([128, h], f32, tag="ps",
                                         name=f"ps2_{rep}_{g}")
                        mm_early(ps2, ttT, g, 2)
                        ps3 = ppool.tile([128, h], f32, tag="ps",
                                         name=f"ps3_{rep}_{g}")
                        mm_early(ps3, ttT, g, 3)
                        ps_of[g] = (ps2, ps3)
                    pending.append((g, Rb, ttT))
                    if len(pending) > LA:
                        run_finalize(pending.pop(0))
                for entry in pending:
                    run_finalize(entry)
                # the next table's AllGather: exposed at the layer boundary
                if L < 2:
                    emit_ag(rep, L + 1)
                return next_tt

            for rep in range(repeat):
                if rep == 0:
                    xall = xall0
                else:
                    xall = xpool.tile([128, NT, h], bf16, tag="xall",
                                      name=f"xall{rep}")
                    nc.sync.dma_start(
                        xall[:], t0_in.rearrange("(t p) j -> p t j", p=128)
                    )
                tt_tiles = [xall[:, t, :] for t in range(NT)]
                for L in range(3):
                    tt_tiles = process_layer(rep, L, tt_tiles)

    nc.compile()
    return nc


# -------------------- public entry --------------------
def kernel(x, edge_index, W1, b1, W1_1, b1_1, W2, b2, W3, b3):
    import ml_dtypes
    from concourse.bass_utils import run_bass_kernel_spmd

    bf16 = ml_dtypes.bfloat16
    x = np.asarray(x, dtype=np.float32)
    edge_index = np.asarray(edge_index, dtype=np.int32)
    n_nodes, h = x.shape
    meta = _prep_graph(edge_index, n_nodes, C)
    P, PT, NG, TOT = meta["P"], meta["PT"], meta["NG"], meta["TOT"]

    key = (n_nodes, h, tuple(meta["Kg"]), TOT)
    if key not in _KERNEL_CACHE:
        _KERNEL_CACHE[key] = _build_bass(
            n_nodes, C, h, P, PT, NG, TOT, meta["Kg"], meta["chunks"],
        )
    nc = _KERNEL_CACHE[key]

    oon = meta["orig_of_new"]
    # staged structural constants
    deg_loc = meta["deg_loc"]  # [C, PT] f32, padded with 1.0
    sqd = np.sqrt(deg_loc)  # [C, PT]
    dinv = (1.0 / sqd).astype(np.float32)
    NT = PT // 128
    dinv_nm = np.ascontiguousarray(
        dinv.reshape(C, NT, 128).transpose(0, 2, 1), dtype=np.float32
    )  # [C, 128, NT]
    dinv2_nm = np.ascontiguousarray(dinv_nm * dinv_nm, dtype=np.float32)
    ident = np.eye(128, dtype=np.float32).astype(bf16)

    # layer-0 gather table: bf16(dinv * x) in the region layout, host-staged
    x_new = x[oon]  # [N, h], ordered by new id
    dinv_new = np.concatenate([dinv[c, :P] for c in range(C)])  # [N]
    scaled = (dinv_new[:, None] * x_new).astype(bf16)  # [N, h]
    table0 = np.zeros((NTAB, h), dtype=bf16)
    table0[_trow(np.arange(n_nodes), P)] = scaled

    Ws = {"W1": W1, "W1_1": W1_1, "W2": W2, "W3": W3}
    bs = {"b1": b1, "b1_1": b1_1, "b2": b2, "b3": b3}
    # weights pre-arranged to the PE lhs layout [(c p) j -> p c j] in bf16
    Wstage = {
        k: np.ascontiguousarray(
            np.asarray(v, dtype=np.float32)
            .reshape(2, 128, h)
            .transpose(1, 0, 2)
            .astype(bf16)
        )
        for k, v in Ws.items()
    }
    in_maps = []
    for c in range(C):
        t0 = np.zeros((PT, h), dtype=bf16)
        t0[:P] = scaled[c * P : (c + 1) * P]
        m = {
            "t0": t0,
            "table0": table0,
            "gidx": np.ascontiguousarray(meta["gidx"][c]),
            "dinv_nm": dinv_nm[c],
            "dinv2_nm": dinv2_nm[c],
            "sqd_row": np.ascontiguousarray(sqd[c], dtype=np.float32),
            "ident": ident,
        }
        for k, v in Wstage.items():
            m[k] = v
        for k, v in bs.items():
            m[k] = np.ascontiguousarray(v, dtype=np.float32)
        in_maps.append(m)

    global LAST_RESULTS
    LAST_RESULTS = run_bass_kernel_spmd(nc, in_maps, core_ids=list(range(C)))
    res = LAST_RESULTS.results

    out2_new = np.concatenate(
        [np.asarray(res[c]["out23"][0]) for c in range(C)], axis=0
    ).astype(np.float32)
    out3_new = np.concatenate(
        [np.asarray(res[c]["out23"][1]) for c in range(C)], axis=0
    ).astype(np.float32)
    new_id = meta["new_id"]
    return out2_new[new_id], out3_new[new_id]
